# revision 18
# baseline (speedup 1.0000x reference)
"""TRN2 Bass kernel for nn_NeuralNetwork_48576080117816 (dense MLP with
Toeplitz-parametrized first layer).

  q     = relu(concat(x_frame, h_esn) @ toeplitz(W1).T + b1)   [B, 1024]
  slope = tanh(q @ W_slope.T + b_slope)                        [B, 64]
  intcp = q @ W_int.T + b_int                                  [B, 64]

Strategy: data-parallel over batch across 8 cores (8192 rows each), weights
replicated. All tensors are staged on host in feature-major (transposed)
layout so the contraction dim lands on SBUF partitions with no on-chip
transposes:

  xT   [1024, B_loc]  combined input, transposed
  w1tT [1024, 1024]   toeplitz(W1).T  (k on rows, n on cols)
  wsi  [1024, 128]    concat(W_slope.T, W_int.T) -> one fused second matmul
  outT [128, B_loc]   rows 0:64 = slope.T (pre-transpose), 64:128 = intcp.T

Matmuls run in float32r (fp32 storage, ~bf16-pair replay on the PE): measured
227 ns per 128x128x512 matmul (vs 215 bf16) with 1.5e-4 rel error per
K=1024 contraction. Per-core PE floor ~= 1152 matmuls * 227 ns ~= 262 us.
"""

import numpy as np

import concourse.bacc as bacc
import concourse.mybir as mybir
import concourse.tile as tile
from concourse import bass_utils

B = 65536
N_CORES = 8
B_LOC = B // N_CORES          # 8192 rows per core
FRAME, ESN, LAST = 64, 960, 1024
COMB = FRAME + ESN            # 1024, contraction dim of matmul 1
KC = COMB // 128              # 8 k-chunks
NC_ = LAST // 128             # 8 n-chunks
BLK = 512                     # batch columns per block (PSUM bank = 512 f32)
NBLK = B_LOC // BLK           # 16 blocks per core

F32 = mybir.dt.float32
MMDT = mybir.dt.float32r

_CACHE = {}


def _build():
    if "nc" in _CACHE:
        return _CACHE["nc"]
    nc = bacc.Bacc("TRN2", target_bir_lowering=False, debug=False)

    xT_d = nc.dram_tensor("xT", [COMB, B_LOC], MMDT, kind="ExternalInput")
    # Toeplitz first layer: stationary tile for (k, n) depends only on the
    # diagonal d = k - n + 7, so only 15 distinct 128x128 tiles exist.
    w1_d = nc.dram_tensor("w1diag", [128, 15, 128], MMDT, kind="ExternalInput")
    wsi_d = nc.dram_tensor("wsi", [LAST, 128], MMDT, kind="ExternalInput")
    b1_d = nc.dram_tensor("b1t", [128, NC_], F32, kind="ExternalInput")
    bsi_d = nc.dram_tensor("bsi", [128, 1], F32, kind="ExternalInput")
    out_d = nc.dram_tensor("outT", [128, B_LOC], F32, kind="ExternalOutput")

    xT_r = xT_d.ap().rearrange("(k p) b -> p k b", p=128)
    wsi_r = wsi_d.ap().rearrange("(c p) m -> p c m", p=128)

    with tile.TileContext(nc) as tc:
        with (
            tc.tile_pool(name="consts", bufs=1) as consts,
            tc.tile_pool(name="xp", bufs=3) as xp,
            tc.tile_pool(name="qp", bufs=3) as qp,
            tc.tile_pool(name="op", bufs=3) as op,
            tc.tile_pool(name="psq", bufs=6, space="PSUM") as psq,
            tc.tile_pool(name="pso", bufs=2, space="PSUM") as pso,
        ):
            w1_sb = consts.tile([128, 15, 128], MMDT)
            wsi_sb = consts.tile([128, KC, 128], MMDT)
            b1_sb = consts.tile([128, NC_], F32)
            bsi_sb = consts.tile([128, 1], F32)
            warm = consts.tile([128, BLK], mybir.dt.bfloat16)
            nc.vector.memset(warm, 0.0)
            nc.sync.dma_start(out=b1_sb, in_=b1_d.ap())
            nc.sync.dma_start(out=bsi_sb, in_=bsi_d.ap())
            # Block 0 inputs, issued interleaved with the weight diagonals in
            # first-use order (group n=0 uses diagonal d=k+7 with x chunk k),
            # so the first matmul gate is ~300KB of DMA and each following
            # chunk lands just ahead of its matmul.
            xt0 = xp.tile([128, KC, BLK], MMDT, tag="xt")
            for k in range(KC):
                nc.sync.dma_start(out=w1_sb[:, k + 7, :], in_=w1_d.ap()[:, k + 7, :])
                nc.sync.dma_start(out=xt0[:, k, :], in_=xT_r[:, k, 0:BLK])
            for dd in range(6, -1, -1):
                nc.sync.dma_start(out=w1_sb[:, dd, :], in_=w1_d.ap()[:, dd, :])
            nc.sync.dma_start(out=wsi_sb, in_=wsi_r)

            # Warm up the PE (HAM clock gate) with dummy matmuls on the
            # zeroed tile while the first DMAs are still in flight.
            wsc = op.tile([128, 1], F32, tag="warmsink")

            def warm_mm(count):
                for _ in range(count):
                    pw = psq.tile([128, 256], F32, tag="pq")
                    nc.tensor.matmul(pw, warm[:, 0:128], warm[:, 0:256],
                                     start=True, stop=True)
                    _CACHE["last_warm"] = pw

            warm_mm(24)

            def phase1(blk):
                bs = slice(blk * BLK, (blk + 1) * BLK)
                if blk == 0:
                    xt = xt0
                else:
                    xt = xp.tile([128, KC, BLK], MMDT, tag="xt")
                    nc.sync.dma_start(out=xt, in_=xT_r[:, :, bs])

                qt = qp.tile([128, NC_, BLK], MMDT, tag="qt")
                for n in range(NC_):
                    pq = psq.tile([128, BLK], F32, tag="pq")
                    for k in range(KC):
                        nc.tensor.matmul(
                            pq,
                            w1_sb[:, k - n + 7, :],
                            xt[:, k, :],
                            start=(k == 0),
                            stop=(k == KC - 1),
                        )
                        # Block 0 is DMA-paced (weights+x still arriving);
                        # keep the PE continuously busy so the HAM clock
                        # gate stays open through the window.
                        if blk == 0 and n < 2:
                            warm_mm(4 if n == 0 else 2)
                    if blk == 0 and n == 1:
                        nc.vector.tensor_copy(wsc, _CACHE["last_warm"][:, 0:1])
                    # relu(x + b1), alternating engines so neither stalls PE
                    if n % 2 == 0:
                        nc.scalar.activation(
                            qt[:, n, :], pq,
                            mybir.ActivationFunctionType.Relu,
                            bias=b1_sb[:, n:n + 1],
                        )
                    else:
                        nc.vector.tensor_scalar(
                            out=qt[:, n, :], in0=pq,
                            scalar1=b1_sb[:, n:n + 1], scalar2=0.0,
                            op0=mybir.AluOpType.add, op1=mybir.AluOpType.max,
                        )
                return qt

            def phase2(blk, qt, nsplit=1):
                w = BLK // nsplit
                for s in range(nsplit):
                    lo = blk * BLK + s * w
                    po = pso.tile([128, w], F32, tag="po")
                    for c in range(KC):
                        nc.tensor.matmul(
                            po, wsi_sb[:, c, :], qt[:, c, s * w:(s + 1) * w],
                            start=(c == 0), stop=(c == KC - 1),
                        )
                    ot = op.tile([128, w], F32, tag="ot")
                    nc.scalar.activation(
                        ot[0:64, :], po[0:64, :],
                        mybir.ActivationFunctionType.Tanh,
                        bias=bsi_sb[0:64, :],
                    )
                    nc.vector.tensor_scalar_add(ot[64:128, :], po[64:128, :],
                                                bsi_sb[64:128, :])
                    nc.sync.dma_start(out=out_d.ap()[:, lo:lo + w], in_=ot)

            # Software pipeline: emit block b's phase-2 after block b+1's
            # phase-1 so the PE never waits on the relu chain at block
            # boundaries.
            prev = None
            for blk in range(NBLK):
                qt = phase1(blk)
                if prev is not None:
                    phase2(*prev)
                prev = (blk, qt)
            phase2(*prev, nsplit=4)

    nc.compile()
    _CACHE["nc"] = nc
    return nc


def _toeplitz(W):
    n_rows, n_cols = W.shape
    params = np.concatenate([W[::-1, 0], W[0, 1:]])
    idx = (n_rows - 1) - np.arange(n_rows)[:, None] + np.arange(n_cols)[None, :]
    return params[idx]


def _prep_inputs(x_frame, h_esn, W1, b1, W_slope, b_slope, W_int, b_int):
    xT = np.ascontiguousarray(
        np.concatenate([x_frame, h_esn], axis=1).T.astype(np.float32))
    # w1diag[p, d, j] = toeplitz(W1).T[k*128+p, n*128+j] for d = k-n+7
    #                 = params[1023 + (d-7)*128 + p - j]
    params = np.concatenate([W1[::-1, 0], W1[0, 1:]]).astype(np.float32)
    idx = (1023 + (np.arange(15)[None, :, None] - 7) * 128
           + np.arange(128)[:, None, None] - np.arange(128)[None, None, :])
    w1diag = np.ascontiguousarray(params[idx])
    wsi = np.ascontiguousarray(
        np.concatenate([W_slope.T, W_int.T], axis=1).astype(np.float32))
    b1t = np.ascontiguousarray(b1.reshape(NC_, 128).T.astype(np.float32))
    bsi = np.ascontiguousarray(
        np.concatenate([b_slope, b_int])[:, None].astype(np.float32))
    in_maps = []
    for c in range(N_CORES):
        in_maps.append({
            "xT": np.ascontiguousarray(xT[:, c * B_LOC:(c + 1) * B_LOC]),
            "w1diag": w1diag,
            "wsi": wsi,
            "b1t": b1t,
            "bsi": bsi,
        })
    return in_maps


def _run(inputs, trace=False, **trace_kwargs):
    nc = _build()
    in_maps = _prep_inputs(**inputs)
    res = bass_utils.run_bass_kernel_spmd(
        nc, in_maps, core_ids=list(range(N_CORES)), trace=trace, **trace_kwargs)
    slope = np.empty((B, FRAME), np.float32)
    intercept = np.empty((B, FRAME), np.float32)
    for c in range(N_CORES):
        outT = res.results[c]["outT"]
        slope[c * B_LOC:(c + 1) * B_LOC] = outT[0:64].T
        intercept[c * B_LOC:(c + 1) * B_LOC] = outT[64:128].T
    return (slope, intercept), res


def kernel(**inputs):
    outs, _ = _run(inputs, trace=False)
    return outs


# revision 22
# speedup vs baseline: 1.0100x; 1.0100x over previous
"""TRN2 Bass kernel for nn_NeuralNetwork_48576080117816 (dense MLP with
Toeplitz-parametrized first layer).

  q     = relu(concat(x_frame, h_esn) @ toeplitz(W1).T + b1)   [B, 1024]
  slope = tanh(q @ W_slope.T + b_slope)                        [B, 64]
  intcp = q @ W_int.T + b_int                                  [B, 64]

Strategy: data-parallel over batch across 8 cores (8192 rows each), weights
replicated. All tensors are staged on host in feature-major (transposed)
layout so the contraction dim lands on SBUF partitions with no on-chip
transposes:

  xT   [1024, B_loc]  combined input, transposed
  w1tT [1024, 1024]   toeplitz(W1).T  (k on rows, n on cols)
  wsi  [1024, 128]    concat(W_slope.T, W_int.T) -> one fused second matmul
  outT [128, B_loc]   rows 0:64 = slope.T (pre-transpose), 64:128 = intcp.T

Matmuls run in float32r (fp32 storage, ~bf16-pair replay on the PE): measured
227 ns per 128x128x512 matmul (vs 215 bf16) with 1.5e-4 rel error per
K=1024 contraction. Per-core PE floor ~= 1152 matmuls * 227 ns ~= 262 us.
"""

import numpy as np

import concourse.bacc as bacc
import concourse.mybir as mybir
import concourse.tile as tile
from concourse import bass_utils

B = 65536
N_CORES = 8
B_LOC = B // N_CORES          # 8192 rows per core
FRAME, ESN, LAST = 64, 960, 1024
COMB = FRAME + ESN            # 1024, contraction dim of matmul 1
KC = COMB // 128              # 8 k-chunks
NC_ = LAST // 128             # 8 n-chunks
BLK = 512                     # batch columns per block (PSUM bank = 512 f32)
NBLK = B_LOC // BLK           # 16 blocks per core

F32 = mybir.dt.float32
MMDT = mybir.dt.float32r

_CACHE = {}


def _build():
    if "nc" in _CACHE:
        return _CACHE["nc"]
    nc = bacc.Bacc("TRN2", target_bir_lowering=False, debug=False)

    xT_d = nc.dram_tensor("xT", [COMB, B_LOC], MMDT, kind="ExternalInput")
    # Toeplitz first layer: stationary tile for (k, n) depends only on the
    # diagonal d = k - n + 7, so only 15 distinct 128x128 tiles exist.
    w1_d = nc.dram_tensor("w1diag", [128, 15, 128], MMDT, kind="ExternalInput")
    wsi_d = nc.dram_tensor("wsi", [LAST, 128], MMDT, kind="ExternalInput")
    b1_d = nc.dram_tensor("b1t", [128, NC_], F32, kind="ExternalInput")
    bsi_d = nc.dram_tensor("bsi", [128, 1], F32, kind="ExternalInput")
    out_d = nc.dram_tensor("outT", [128, B_LOC], F32, kind="ExternalOutput")

    xT_r = xT_d.ap().rearrange("(k p) b -> p k b", p=128)
    wsi_r = wsi_d.ap().rearrange("(c p) m -> p c m", p=128)

    with tile.TileContext(nc) as tc:
        with (
            tc.tile_pool(name="consts", bufs=1) as consts,
            tc.tile_pool(name="xp", bufs=3) as xp,
            tc.tile_pool(name="qp", bufs=3) as qp,
            tc.tile_pool(name="op", bufs=3) as op,
            tc.tile_pool(name="psq", bufs=6, space="PSUM") as psq,
            tc.tile_pool(name="pso", bufs=2, space="PSUM") as pso,
        ):
            w1_sb = consts.tile([128, 15, 128], MMDT)
            wsi_sb = consts.tile([128, KC, 128], MMDT)
            b1_sb = consts.tile([128, NC_], F32)
            bsi_sb = consts.tile([128, 1], F32)
            warm = consts.tile([128, BLK], mybir.dt.bfloat16)
            nc.vector.memset(warm, 0.0)
            nc.sync.dma_start(out=b1_sb, in_=b1_d.ap())
            nc.sync.dma_start(out=bsi_sb, in_=bsi_d.ap())
            # Block 0 inputs, issued interleaved with the weight diagonals in
            # first-use order (group n=0 uses diagonal d=k+7 with x chunk k),
            # so the first matmul gate is ~300KB of DMA and each following
            # chunk lands just ahead of its matmul.
            xt0 = xp.tile([128, KC, BLK], MMDT, tag="xt")
            for dd in range(4, 15):
                nc.sync.dma_start(out=w1_sb[:, dd, :], in_=w1_d.ap()[:, dd, :])
            for k in range(KC):
                nc.sync.dma_start(out=xt0[:, k, :], in_=xT_r[:, k, 0:BLK])
            for dd in range(3, -1, -1):
                nc.sync.dma_start(out=w1_sb[:, dd, :], in_=w1_d.ap()[:, dd, :])
            nc.sync.dma_start(out=wsi_sb, in_=wsi_r)

            # Warm up the PE (HAM clock gate) with dummy matmuls on the
            # zeroed tile while the first DMAs are still in flight.
            wsc = op.tile([128, 1], F32, tag="warmsink")

            def warm_mm(count):
                for _ in range(count):
                    pw = psq.tile([128, 256], F32, tag="pq")
                    nc.tensor.matmul(pw, warm[:, 0:128], warm[:, 0:256],
                                     start=True, stop=True)
                    _CACHE["last_warm"] = pw

            warm_mm(16)

            def phase1(blk):
                bs = slice(blk * BLK, (blk + 1) * BLK)
                if blk == 0:
                    xt = xt0
                else:
                    xt = xp.tile([128, KC, BLK], MMDT, tag="xt")
                    nc.sync.dma_start(out=xt, in_=xT_r[:, :, bs])

                qt = qp.tile([128, NC_, BLK], MMDT, tag="qt")

                def relu(n, pq):
                    # relu(x + b1), alternating engines so neither stalls PE
                    if n % 2 == 0:
                        nc.scalar.activation(
                            qt[:, n, :], pq,
                            mybir.ActivationFunctionType.Relu,
                            bias=b1_sb[:, n:n + 1],
                        )
                    else:
                        nc.vector.tensor_scalar(
                            out=qt[:, n, :], in0=pq,
                            scalar1=b1_sb[:, n:n + 1], scalar2=0.0,
                            op0=mybir.AluOpType.add, op1=mybir.AluOpType.max,
                        )

                if blk == 0:
                    # Block 0 is DMA-paced (weights + x chunks still arriving)
                    # so run k-outer with 4 concurrent PSUM groups: each
                    # arriving x chunk immediately feeds 4 matmuls, keeping
                    # the PE (and the HAM clock gate) busy through the
                    # window. Two passes of 4 n-groups (PSUM has 8 banks).
                    for half in range(2):
                        ns = range(4 * half, 4 * half + 4)
                        pqs = {n: psq.tile([128, BLK], F32, tag="pq",
                                           name=f"pq0_{n}")
                               for n in ns}
                        for k in range(KC):
                            for n in ns:
                                nc.tensor.matmul(
                                    pqs[n],
                                    w1_sb[:, k - n + 7, :],
                                    xt[:, k, :],
                                    start=(k == 0),
                                    stop=(k == KC - 1),
                                )
                            if half == 0:
                                warm_mm(1)
                        for n in ns:
                            relu(n, pqs[n])
                    nc.vector.tensor_copy(wsc, _CACHE["last_warm"][:, 0:1])
                else:
                    for n in range(NC_):
                        pq = psq.tile([128, BLK], F32, tag="pq")
                        for k in range(KC):
                            nc.tensor.matmul(
                                pq,
                                w1_sb[:, k - n + 7, :],
                                xt[:, k, :],
                                start=(k == 0),
                                stop=(k == KC - 1),
                            )
                        relu(n, pq)
                return qt

            def phase2(blk, qt, nsplit=1):
                w = BLK // nsplit
                for s in range(nsplit):
                    lo = blk * BLK + s * w
                    po = pso.tile([128, w], F32, tag="po")
                    for c in range(KC):
                        nc.tensor.matmul(
                            po, wsi_sb[:, c, :], qt[:, c, s * w:(s + 1) * w],
                            start=(c == 0), stop=(c == KC - 1),
                        )
                    ot = op.tile([128, w], F32, tag="ot")
                    nc.scalar.activation(
                        ot[0:64, :], po[0:64, :],
                        mybir.ActivationFunctionType.Tanh,
                        bias=bsi_sb[0:64, :],
                    )
                    nc.vector.tensor_scalar_add(ot[64:128, :], po[64:128, :],
                                                bsi_sb[64:128, :])
                    nc.sync.dma_start(out=out_d.ap()[:, lo:lo + w], in_=ot)

            # Software pipeline: emit block b's phase-2 after block b+1's
            # phase-1 so the PE never waits on the relu chain at block
            # boundaries.
            prev = None
            for blk in range(NBLK):
                qt = phase1(blk)
                if prev is not None:
                    phase2(*prev)
                prev = (blk, qt)
            phase2(*prev, nsplit=4)

    nc.compile()
    _CACHE["nc"] = nc
    return nc


def _toeplitz(W):
    n_rows, n_cols = W.shape
    params = np.concatenate([W[::-1, 0], W[0, 1:]])
    idx = (n_rows - 1) - np.arange(n_rows)[:, None] + np.arange(n_cols)[None, :]
    return params[idx]


def _prep_inputs(x_frame, h_esn, W1, b1, W_slope, b_slope, W_int, b_int):
    xT = np.ascontiguousarray(
        np.concatenate([x_frame, h_esn], axis=1).T.astype(np.float32))
    # w1diag[p, d, j] = toeplitz(W1).T[k*128+p, n*128+j] for d = k-n+7
    #                 = params[1023 + (d-7)*128 + p - j]
    params = np.concatenate([W1[::-1, 0], W1[0, 1:]]).astype(np.float32)
    idx = (1023 + (np.arange(15)[None, :, None] - 7) * 128
           + np.arange(128)[:, None, None] - np.arange(128)[None, None, :])
    w1diag = np.ascontiguousarray(params[idx])
    wsi = np.ascontiguousarray(
        np.concatenate([W_slope.T, W_int.T], axis=1).astype(np.float32))
    b1t = np.ascontiguousarray(b1.reshape(NC_, 128).T.astype(np.float32))
    bsi = np.ascontiguousarray(
        np.concatenate([b_slope, b_int])[:, None].astype(np.float32))
    in_maps = []
    for c in range(N_CORES):
        in_maps.append({
            "xT": np.ascontiguousarray(xT[:, c * B_LOC:(c + 1) * B_LOC]),
            "w1diag": w1diag,
            "wsi": wsi,
            "b1t": b1t,
            "bsi": bsi,
        })
    return in_maps


def _run(inputs, trace=False, **trace_kwargs):
    nc = _build()
    in_maps = _prep_inputs(**inputs)
    res = bass_utils.run_bass_kernel_spmd(
        nc, in_maps, core_ids=list(range(N_CORES)), trace=trace, **trace_kwargs)
    slope = np.empty((B, FRAME), np.float32)
    intercept = np.empty((B, FRAME), np.float32)
    for c in range(N_CORES):
        outT = res.results[c]["outT"]
        slope[c * B_LOC:(c + 1) * B_LOC] = outT[0:64].T
        intercept[c * B_LOC:(c + 1) * B_LOC] = outT[64:128].T
    return (slope, intercept), res


def kernel(**inputs):
    outs, _ = _run(inputs, trace=False)
    return outs


# revision 23
# speedup vs baseline: 1.0265x; 1.0163x over previous
"""TRN2 Bass kernel for nn_NeuralNetwork_48576080117816 (dense MLP with
Toeplitz-parametrized first layer).

  q     = relu(concat(x_frame, h_esn) @ toeplitz(W1).T + b1)   [B, 1024]
  slope = tanh(q @ W_slope.T + b_slope)                        [B, 64]
  intcp = q @ W_int.T + b_int                                  [B, 64]

Strategy: data-parallel over batch across 8 cores (8192 rows each), weights
replicated. All tensors are staged on host in feature-major (transposed)
layout so the contraction dim lands on SBUF partitions with no on-chip
transposes:

  xT   [1024, B_loc]  combined input, transposed
  w1tT [1024, 1024]   toeplitz(W1).T  (k on rows, n on cols)
  wsi  [1024, 128]    concat(W_slope.T, W_int.T) -> one fused second matmul
  outT [128, B_loc]   rows 0:64 = slope.T (pre-transpose), 64:128 = intcp.T

Matmuls run in float32r (fp32 storage, ~bf16-pair replay on the PE): measured
227 ns per 128x128x512 matmul (vs 215 bf16) with 1.5e-4 rel error per
K=1024 contraction. Per-core PE floor ~= 1152 matmuls * 227 ns ~= 262 us.
"""

import numpy as np

import concourse.bacc as bacc
import concourse.mybir as mybir
import concourse.tile as tile
from concourse import bass_utils

B = 65536
N_CORES = 8
B_LOC = B // N_CORES          # 8192 rows per core
FRAME, ESN, LAST = 64, 960, 1024
COMB = FRAME + ESN            # 1024, contraction dim of matmul 1
KC = COMB // 128              # 8 k-chunks
NC_ = LAST // 128             # 8 n-chunks
BLK = 512                     # batch columns per block (PSUM bank = 512 f32)
NBLK = B_LOC // BLK           # 16 blocks per core

F32 = mybir.dt.float32
MMDT = mybir.dt.float32r

_CACHE = {}


def _build():
    if "nc" in _CACHE:
        return _CACHE["nc"]
    nc = bacc.Bacc("TRN2", target_bir_lowering=False, debug=False)

    xT_d = nc.dram_tensor("xT", [COMB, B_LOC], MMDT, kind="ExternalInput")
    # Toeplitz first layer: stationary tile for (k, n) depends only on the
    # diagonal d = k - n + 7, so only 15 distinct 128x128 tiles exist.
    w1_d = nc.dram_tensor("w1diag", [128, 15, 128], MMDT, kind="ExternalInput")
    wsi_d = nc.dram_tensor("wsi", [LAST, 128], MMDT, kind="ExternalInput")
    b1_d = nc.dram_tensor("b1t", [128, NC_], F32, kind="ExternalInput")
    bsi_d = nc.dram_tensor("bsi", [128, 1], F32, kind="ExternalInput")
    out_d = nc.dram_tensor("outT", [128, B_LOC], F32, kind="ExternalOutput")

    xT_r = xT_d.ap().rearrange("(k p) b -> p k b", p=128)
    wsi_r = wsi_d.ap().rearrange("(c p) m -> p c m", p=128)

    with tile.TileContext(nc) as tc:
        with (
            tc.tile_pool(name="consts", bufs=1) as consts,
            tc.tile_pool(name="xp", bufs=3) as xp,
            tc.tile_pool(name="qp", bufs=3) as qp,
            tc.tile_pool(name="op", bufs=3) as op,
            tc.tile_pool(name="psq", bufs=6, space="PSUM") as psq,
            tc.tile_pool(name="pso", bufs=2, space="PSUM") as pso,
        ):
            w1_sb = consts.tile([128, 15, 128], MMDT)
            wsi_sb = consts.tile([128, KC, 128], MMDT)
            b1_sb = consts.tile([128, NC_], F32)
            bsi_sb = consts.tile([128, 1], F32)
            warm = consts.tile([128, BLK], mybir.dt.bfloat16)
            nc.vector.memset(warm, 0.0)
            nc.sync.dma_start(out=b1_sb, in_=b1_d.ap())
            nc.sync.dma_start(out=bsi_sb, in_=bsi_d.ap())
            # Block 0 inputs, issued interleaved with the weight diagonals in
            # first-use order (group n=0 uses diagonal d=k+7 with x chunk k),
            # so the first matmul gate is ~300KB of DMA and each following
            # chunk lands just ahead of its matmul.
            xt0 = xp.tile([128, KC, BLK], MMDT, tag="xt")
            nc.sync.dma_start(out=w1_sb[:, 4:15, :], in_=w1_d.ap()[:, 4:15, :])
            for k in range(KC):
                nc.sync.dma_start(out=xt0[:, k, :], in_=xT_r[:, k, 0:BLK])
            nc.sync.dma_start(out=w1_sb[:, 0:4, :], in_=w1_d.ap()[:, 0:4, :])
            nc.sync.dma_start(out=wsi_sb, in_=wsi_r)

            # Warm up the PE (HAM clock gate) with dummy matmuls on the
            # zeroed tile while the first DMAs are still in flight.
            wsc = op.tile([128, 1], F32, tag="warmsink")

            def warm_mm(count):
                for _ in range(count):
                    pw = psq.tile([128, 256], F32, tag="pq")
                    nc.tensor.matmul(pw, warm[:, 0:128], warm[:, 0:256],
                                     start=True, stop=True)
                    _CACHE["last_warm"] = pw

            warm_mm(16)

            def phase1(blk):
                bs = slice(blk * BLK, (blk + 1) * BLK)
                if blk == 0:
                    xt = xt0
                else:
                    xt = xp.tile([128, KC, BLK], MMDT, tag="xt")
                    nc.sync.dma_start(out=xt, in_=xT_r[:, :, bs])

                qt = qp.tile([128, NC_, BLK], MMDT, tag="qt")

                def relu(n, pq):
                    # relu(x + b1), alternating engines so neither stalls PE
                    if n % 2 == 0:
                        nc.scalar.activation(
                            qt[:, n, :], pq,
                            mybir.ActivationFunctionType.Relu,
                            bias=b1_sb[:, n:n + 1],
                        )
                    else:
                        nc.vector.tensor_scalar(
                            out=qt[:, n, :], in0=pq,
                            scalar1=b1_sb[:, n:n + 1], scalar2=0.0,
                            op0=mybir.AluOpType.add, op1=mybir.AluOpType.max,
                        )

                if blk == 0:
                    # Block 0 is DMA-paced (weights + x chunks still arriving)
                    # so run k-outer with 4 concurrent PSUM groups: each
                    # arriving x chunk immediately feeds 4 matmuls, keeping
                    # the PE (and the HAM clock gate) busy through the
                    # window. Two passes of 4 n-groups (PSUM has 8 banks).
                    for half in range(2):
                        ns = range(4 * half, 4 * half + 4)
                        pqs = {n: psq.tile([128, BLK], F32, tag="pq",
                                           name=f"pq0_{n}")
                               for n in ns}
                        for k in range(KC):
                            for n in ns:
                                nc.tensor.matmul(
                                    pqs[n],
                                    w1_sb[:, k - n + 7, :],
                                    xt[:, k, :],
                                    start=(k == 0),
                                    stop=(k == KC - 1),
                                )
                            if half == 0:
                                warm_mm(1)
                        for n in ns:
                            relu(n, pqs[n])
                    nc.vector.tensor_copy(wsc, _CACHE["last_warm"][:, 0:1])
                else:
                    for n in range(NC_):
                        pq = psq.tile([128, BLK], F32, tag="pq")
                        for k in range(KC):
                            nc.tensor.matmul(
                                pq,
                                w1_sb[:, k - n + 7, :],
                                xt[:, k, :],
                                start=(k == 0),
                                stop=(k == KC - 1),
                            )
                        relu(n, pq)
                return qt

            def phase2(blk, qt, nsplit=1):
                w = BLK // nsplit
                for s in range(nsplit):
                    lo = blk * BLK + s * w
                    po = pso.tile([128, w], F32, tag="po")
                    for c in range(KC):
                        nc.tensor.matmul(
                            po, wsi_sb[:, c, :], qt[:, c, s * w:(s + 1) * w],
                            start=(c == 0), stop=(c == KC - 1),
                        )
                    ot = op.tile([128, w], F32, tag="ot")
                    nc.scalar.activation(
                        ot[0:64, :], po[0:64, :],
                        mybir.ActivationFunctionType.Tanh,
                        bias=bsi_sb[0:64, :],
                    )
                    nc.vector.tensor_scalar_add(ot[64:128, :], po[64:128, :],
                                                bsi_sb[64:128, :])
                    nc.sync.dma_start(out=out_d.ap()[:, lo:lo + w], in_=ot)

            # Software pipeline: emit block b's phase-2 after block b+1's
            # phase-1 so the PE never waits on the relu chain at block
            # boundaries.
            prev = None
            for blk in range(NBLK):
                qt = phase1(blk)
                if prev is not None:
                    phase2(*prev)
                prev = (blk, qt)
            phase2(*prev, nsplit=4)

    nc.compile()
    _CACHE["nc"] = nc
    return nc


def _toeplitz(W):
    n_rows, n_cols = W.shape
    params = np.concatenate([W[::-1, 0], W[0, 1:]])
    idx = (n_rows - 1) - np.arange(n_rows)[:, None] + np.arange(n_cols)[None, :]
    return params[idx]


def _prep_inputs(x_frame, h_esn, W1, b1, W_slope, b_slope, W_int, b_int):
    xT = np.ascontiguousarray(
        np.concatenate([x_frame, h_esn], axis=1).T.astype(np.float32))
    # w1diag[p, d, j] = toeplitz(W1).T[k*128+p, n*128+j] for d = k-n+7
    #                 = params[1023 + (d-7)*128 + p - j]
    params = np.concatenate([W1[::-1, 0], W1[0, 1:]]).astype(np.float32)
    idx = (1023 + (np.arange(15)[None, :, None] - 7) * 128
           + np.arange(128)[:, None, None] - np.arange(128)[None, None, :])
    w1diag = np.ascontiguousarray(params[idx])
    wsi = np.ascontiguousarray(
        np.concatenate([W_slope.T, W_int.T], axis=1).astype(np.float32))
    b1t = np.ascontiguousarray(b1.reshape(NC_, 128).T.astype(np.float32))
    bsi = np.ascontiguousarray(
        np.concatenate([b_slope, b_int])[:, None].astype(np.float32))
    in_maps = []
    for c in range(N_CORES):
        in_maps.append({
            "xT": np.ascontiguousarray(xT[:, c * B_LOC:(c + 1) * B_LOC]),
            "w1diag": w1diag,
            "wsi": wsi,
            "b1t": b1t,
            "bsi": bsi,
        })
    return in_maps


def _run(inputs, trace=False, **trace_kwargs):
    nc = _build()
    in_maps = _prep_inputs(**inputs)
    res = bass_utils.run_bass_kernel_spmd(
        nc, in_maps, core_ids=list(range(N_CORES)), trace=trace, **trace_kwargs)
    slope = np.empty((B, FRAME), np.float32)
    intercept = np.empty((B, FRAME), np.float32)
    for c in range(N_CORES):
        outT = res.results[c]["outT"]
        slope[c * B_LOC:(c + 1) * B_LOC] = outT[0:64].T
        intercept[c * B_LOC:(c + 1) * B_LOC] = outT[64:128].T
    return (slope, intercept), res


def kernel(**inputs):
    outs, _ = _run(inputs, trace=False)
    return outs


# revision 28
# speedup vs baseline: 1.0359x; 1.0092x over previous
"""TRN2 Bass kernel for nn_NeuralNetwork_48576080117816 (dense MLP with
Toeplitz-parametrized first layer).

  q     = relu(concat(x_frame, h_esn) @ toeplitz(W1).T + b1)   [B, 1024]
  slope = tanh(q @ W_slope.T + b_slope)                        [B, 64]
  intcp = q @ W_int.T + b_int                                  [B, 64]

Strategy: data-parallel over batch across 8 cores (8192 rows each), weights
replicated. All tensors are staged on host in feature-major (transposed)
layout so the contraction dim lands on SBUF partitions with no on-chip
transposes:

  xT   [1024, B_loc]  combined input, transposed
  w1tT [1024, 1024]   toeplitz(W1).T  (k on rows, n on cols)
  wsi  [1024, 128]    concat(W_slope.T, W_int.T) -> one fused second matmul
  outT [128, B_loc]   rows 0:64 = slope.T (pre-transpose), 64:128 = intcp.T

Matmuls run in float32r (fp32 storage, ~bf16-pair replay on the PE): measured
227 ns per 128x128x512 matmul (vs 215 bf16) with 1.5e-4 rel error per
K=1024 contraction. Per-core PE floor ~= 1152 matmuls * 227 ns ~= 262 us.
"""

import numpy as np

import concourse.bacc as bacc
import concourse.mybir as mybir
import concourse.tile as tile
from concourse import bass_utils

B = 65536
N_CORES = 8
B_LOC = B // N_CORES          # 8192 rows per core
FRAME, ESN, LAST = 64, 960, 1024
COMB = FRAME + ESN            # 1024, contraction dim of matmul 1
KC = COMB // 128              # 8 k-chunks
NC_ = LAST // 128             # 8 n-chunks
BLK = 512                     # batch columns per block (PSUM bank = 512 f32)
NBLK = B_LOC // BLK           # 16 blocks per core

F32 = mybir.dt.float32
MMDT = mybir.dt.float32r

_CACHE = {}


def _build():
    if "nc" in _CACHE:
        return _CACHE["nc"]
    nc = bacc.Bacc("TRN2", target_bir_lowering=False, debug=False)

    xT_d = nc.dram_tensor("xT", [COMB, B_LOC], MMDT, kind="ExternalInput")
    # Toeplitz first layer: stationary tile for (k, n) depends only on the
    # diagonal d = k - n + 7, so only 15 distinct 128x128 tiles exist.
    w1_d = nc.dram_tensor("w1diag", [128, 15, 128], MMDT, kind="ExternalInput")
    wsi_d = nc.dram_tensor("wsi", [LAST, 128], MMDT, kind="ExternalInput")
    bias_d = nc.dram_tensor("biases", [128, NC_ + 1], F32, kind="ExternalInput")
    out_d = nc.dram_tensor("outT", [128, B_LOC], F32, kind="ExternalOutput")

    xT_r = xT_d.ap().rearrange("(k p) b -> p k b", p=128)
    wsi_r = wsi_d.ap().rearrange("(c p) m -> p c m", p=128)

    with tile.TileContext(nc) as tc:
        with (
            tc.tile_pool(name="consts", bufs=1) as consts,
            tc.tile_pool(name="xp", bufs=3) as xp,
            tc.tile_pool(name="qp", bufs=3) as qp,
            tc.tile_pool(name="op", bufs=3) as op,
            tc.tile_pool(name="psq", bufs=6, space="PSUM") as psq,
            tc.tile_pool(name="pso", bufs=2, space="PSUM") as pso,
        ):
            w1_sb = consts.tile([128, 15, 128], MMDT)
            wsi_sb = consts.tile([128, KC, 128], MMDT)
            bias_sb = consts.tile([128, NC_ + 1], F32)
            warm = consts.tile([128, BLK], mybir.dt.bfloat16)
            nc.vector.memset(warm, 0.0)
            nc.sync.dma_start(out=bias_sb, in_=bias_d.ap())
            b1_sb = bias_sb[:, 0:NC_]
            bsi_sb = bias_sb[:, NC_:NC_ + 1]
            # Block 0 inputs, issued interleaved with the weight diagonals in
            # first-use order (group n=0 uses diagonal d=k+7 with x chunk k),
            # so the first matmul gate is ~300KB of DMA and each following
            # chunk lands just ahead of its matmul.
            xt0 = xp.tile([128, KC, BLK], MMDT, tag="xt")
            nc.sync.dma_start(out=w1_sb[:, 4:15, :], in_=w1_d.ap()[:, 4:15, :])
            for k in range(KC):
                nc.sync.dma_start(out=xt0[:, k, :], in_=xT_r[:, k, 0:BLK])
            nc.sync.dma_start(out=w1_sb[:, 0:4, :], in_=w1_d.ap()[:, 0:4, :])
            nc.sync.dma_start(out=wsi_sb, in_=wsi_r)

            # Warm up the PE (HAM clock gate) with dummy matmuls on the
            # zeroed tile while the first DMAs are still in flight.
            wsc = op.tile([128, 1], F32, tag="warmsink")

            def warm_mm(count):
                for _ in range(count):
                    pw = psq.tile([128, 256], F32, tag="pq")
                    nc.tensor.matmul(pw, warm[:, 0:128], warm[:, 0:256],
                                     start=True, stop=True)
                    _CACHE["last_warm"] = pw

            warm_mm(28)

            def phase1(blk):
                bs = slice(blk * BLK, (blk + 1) * BLK)
                if blk == 0:
                    xt = xt0
                else:
                    xt = xp.tile([128, KC, BLK], MMDT, tag="xt")
                    nc.sync.dma_start(out=xt, in_=xT_r[:, :, bs])

                qt = qp.tile([128, NC_, BLK], MMDT, tag="qt")

                def relu(n, pq):
                    # relu(x + b1), alternating engines so neither stalls PE
                    if n % 2 == 0:
                        nc.scalar.activation(
                            qt[:, n, :], pq,
                            mybir.ActivationFunctionType.Relu,
                            bias=b1_sb[:, n:n + 1],
                        )
                    else:
                        nc.vector.tensor_scalar(
                            out=qt[:, n, :], in0=pq,
                            scalar1=b1_sb[:, n:n + 1], scalar2=0.0,
                            op0=mybir.AluOpType.add, op1=mybir.AluOpType.max,
                        )

                if blk == 0:
                    # Block 0 is DMA-paced (weights + x chunks still arriving)
                    # so run k-outer with 4 concurrent PSUM groups: each
                    # arriving x chunk immediately feeds 4 matmuls, keeping
                    # the PE (and the HAM clock gate) busy through the
                    # window. Two passes of 4 n-groups (PSUM has 8 banks).
                    for half in range(2):
                        ns = range(4 * half, 4 * half + 4)
                        pqs = {n: psq.tile([128, BLK], F32, tag="pq",
                                           name=f"pq0_{n}")
                               for n in ns}
                        for k in range(KC):
                            for n in ns:
                                nc.tensor.matmul(
                                    pqs[n],
                                    w1_sb[:, k - n + 7, :],
                                    xt[:, k, :],
                                    start=(k == 0),
                                    stop=(k == KC - 1),
                                )
                            if half == 0:
                                warm_mm(1)
                        for n in ns:
                            relu(n, pqs[n])
                    nc.vector.tensor_copy(wsc, _CACHE["last_warm"][:, 0:1])
                else:
                    for n in range(NC_):
                        pq = psq.tile([128, BLK], F32, tag="pq")
                        for k in range(KC):
                            nc.tensor.matmul(
                                pq,
                                w1_sb[:, k - n + 7, :],
                                xt[:, k, :],
                                start=(k == 0),
                                stop=(k == KC - 1),
                            )
                        relu(n, pq)
                return qt

            def phase2(blk, qt, nsplit=1):
                w = BLK // nsplit
                for s in range(nsplit):
                    lo = blk * BLK + s * w
                    po = pso.tile([128, w], F32, tag="po")
                    for c in range(KC):
                        nc.tensor.matmul(
                            po, wsi_sb[:, c, :], qt[:, c, s * w:(s + 1) * w],
                            start=(c == 0), stop=(c == KC - 1),
                        )
                    ot = op.tile([128, w], F32, tag="ot")
                    nc.scalar.activation(
                        ot[0:64, :], po[0:64, :],
                        mybir.ActivationFunctionType.Tanh,
                        bias=bsi_sb[0:64, :],
                    )
                    nc.vector.tensor_scalar_add(ot[64:128, :], po[64:128, :],
                                                bsi_sb[64:128, :])
                    nc.sync.dma_start(out=out_d.ap()[:, lo:lo + w], in_=ot)

            # Software pipeline: emit block b's phase-2 after block b+1's
            # phase-1 so the PE never waits on the relu chain at block
            # boundaries.
            prev = None
            for blk in range(NBLK):
                qt = phase1(blk)
                if prev is not None:
                    phase2(*prev)
                prev = (blk, qt)
            phase2(*prev, nsplit=4)

    nc.compile()
    _CACHE["nc"] = nc
    return nc


def _toeplitz(W):
    n_rows, n_cols = W.shape
    params = np.concatenate([W[::-1, 0], W[0, 1:]])
    idx = (n_rows - 1) - np.arange(n_rows)[:, None] + np.arange(n_cols)[None, :]
    return params[idx]


def _prep_inputs(x_frame, h_esn, W1, b1, W_slope, b_slope, W_int, b_int):
    xT = np.ascontiguousarray(
        np.concatenate([x_frame, h_esn], axis=1).T.astype(np.float32))
    # w1diag[p, d, j] = toeplitz(W1).T[k*128+p, n*128+j] for d = k-n+7
    #                 = params[1023 + (d-7)*128 + p - j]
    params = np.concatenate([W1[::-1, 0], W1[0, 1:]]).astype(np.float32)
    idx = (1023 + (np.arange(15)[None, :, None] - 7) * 128
           + np.arange(128)[:, None, None] - np.arange(128)[None, None, :])
    w1diag = np.ascontiguousarray(params[idx])
    wsi = np.ascontiguousarray(
        np.concatenate([W_slope.T, W_int.T], axis=1).astype(np.float32))
    b1t = b1.reshape(NC_, 128).T.astype(np.float32)
    bsi = np.concatenate([b_slope, b_int])[:, None].astype(np.float32)
    biases = np.ascontiguousarray(np.concatenate([b1t, bsi], axis=1))
    in_maps = []
    for c in range(N_CORES):
        in_maps.append({
            "xT": np.ascontiguousarray(xT[:, c * B_LOC:(c + 1) * B_LOC]),
            "w1diag": w1diag,
            "wsi": wsi,
            "biases": biases,
        })
    return in_maps


def _run(inputs, trace=False, **trace_kwargs):
    nc = _build()
    in_maps = _prep_inputs(**inputs)
    res = bass_utils.run_bass_kernel_spmd(
        nc, in_maps, core_ids=list(range(N_CORES)), trace=trace, **trace_kwargs)
    slope = np.empty((B, FRAME), np.float32)
    intercept = np.empty((B, FRAME), np.float32)
    for c in range(N_CORES):
        outT = res.results[c]["outT"]
        slope[c * B_LOC:(c + 1) * B_LOC] = outT[0:64].T
        intercept[c * B_LOC:(c + 1) * B_LOC] = outT[64:128].T
    return (slope, intercept), res


def kernel(**inputs):
    outs, _ = _run(inputs, trace=False)
    return outs
